# revision 1
# baseline (speedup 1.0000x reference)
"""Conv2d(128->256, 3x3, pad 1) + bias on 16x128x56x56, SPMD over 8 TRN2 cores.

Data-parallel over batch: each core convolves 2 images. Per core the conv is
an implicit GEMM: contraction over CIN=128 (the SBUF partition dim), with the
9 kernel taps accumulated into PSUM via start/stop matmul groups. Each image
is zero-padded to 58x58 in SBUF so every tap is a pure access-pattern shift.
Weights are pre-transposed on the host to [ci, kh, kw, co_tile, co] so all
DMAs are dense. float32r matmuls run at full PE rate for free dim >= 256; we
use N = 448 (8 output rows x 56 cols, one PSUM bank).

HW scheduling notes (measured via hardware-loop differentials):
- HWDGE queues are per issuing engine: inputs ride the ACT queue, outputs
  get the SP queue to themselves, so output drains never stall behind an
  8us image transfer (PSUM banks recycle on time and the PE never starves).
- The padded borders are zeroed once into two persistent buffers at
  startup; per-image work only rewrites the interior, keeping the border
  memsets off the per-image critical path.
- Both image DMA triggers issue before either repack so the ACT engine's
  in-order wait for image 0's data cannot delay image 1's transfer.
- Junk warmup matmuls ramp the PE clock (HAM) while the first image loads.
"""

import numpy as np

B, CIN, COUT, H, W = 16, 128, 256, 56, 56
KH, KW = 3, 3
N_CORES = 8
IMGS_PER_CORE = B // N_CORES  # 2
CO_TILES = COUT // 128  # 2
HP, WP = H + 2, W + 2  # 58, 58
ROWS_PER_BLOCK = 8
N_BLOCKS = H // ROWS_PER_BLOCK  # 7

FIRST_CHUNKS = ((0, 9), (9, 17), (17, 33), (33, H))
WHOLE = ((0, H),)

_COMPILED = {}


def _build(mm_dtype_name: str, repeats: int = 1, n_warmup: int = 40,
           loop_iters: int = 0, ablate: frozenset = frozenset()):
    # ablate flags (timing-only builds): "nomm" (no matmuls), "nodve"
    # (no bias-add drain nor out DMA), "noout" (no out DMA)
    import concourse.bacc as bacc
    import concourse.mybir as mybir
    import concourse.tile as tile

    mm_dt = getattr(mybir.dt, mm_dtype_name)
    f32 = mybir.dt.float32
    u32 = mybir.dt.uint32
    four_byte = mybir.dt.size(mm_dt) == 4
    in_dt = f32 if four_byte else mm_dt
    zero_dt = u32 if four_byte else mybir.dt.uint16

    def mm_view(ap):
        return ap.bitcast(mm_dt) if four_byte else ap

    nc = bacc.Bacc("TRN2", target_bir_lowering=False, debug=False,
                   num_devices=N_CORES)
    x_dram = nc.dram_tensor("x", [IMGS_PER_CORE, CIN, H, W], in_dt,
                            kind="ExternalInput").ap()
    w_dram = nc.dram_tensor("w", [CIN, KH * KW * COUT], in_dt,
                            kind="ExternalInput").ap()
    b_dram = nc.dram_tensor("b", [128, CO_TILES], f32,
                            kind="ExternalInput").ap()
    out_dram = nc.dram_tensor("out", [IMGS_PER_CORE, COUT, H, W], f32,
                              kind="ExternalOutput").ap()

    w_dram_v = mm_view(w_dram).rearrange(
        "c (k t o) -> c k t o", k=KH * KW, t=CO_TILES)

    with tile.TileContext(nc) as tc:
        with (
            tc.tile_pool(name="xp", bufs=1) as x_pool,
            tc.tile_pool(name="st", bufs=2) as stage_pool,
            tc.tile_pool(name="wp", bufs=1) as w_pool,
            tc.tile_pool(name="op", bufs=8) as out_pool,
            tc.tile_pool(name="ps", bufs=7, space="PSUM") as psum_pool,
            tc.tile_pool(name="wups", bufs=1, space="PSUM") as warm_psum_pool,
        ):
            # PE warmup: junk matmuls on a small memset tile ramp the PE
            # clock while the input/weight DMAs are in flight.
            junk = w_pool.tile([128, 128], mm_dt, tag="junk")
            nc.gpsimd.memset(junk[:].bitcast(zero_dt), 0)
            wpsum = warm_psum_pool.tile([128, 64], f32)
            for _ in range(n_warmup):
                nc.tensor.matmul(wpsum[:], junk[:], junk[:, :64], start=True,
                                 stop=True)

            w_sb = w_pool.tile([CIN, KH * KW, CO_TILES, 128], mm_dt)
            b_sb = w_pool.tile([128, CO_TILES], f32, tag="bias")

            # two persistent padded buffers; borders zeroed exactly once
            # (per-image work rewrites only the interior)
            x_pads = []
            for s in range(2):
                xp = x_pool.tile([CIN, HP, WP], mm_dt, tag=f"xpad{s}")
                nc.gpsimd.memset(xp[:, 0, :].bitcast(zero_dt), 0)
                nc.gpsimd.memset(xp[:, HP - 1, :].bitcast(zero_dt), 0)
                nc.gpsimd.memset(xp[:, 1:HP - 1, 0].bitcast(zero_dt), 0)
                nc.gpsimd.memset(xp[:, 1:HP - 1, WP - 1].bitcast(zero_dt), 0)
                x_pads.append(xp)

            def trigger_load(img, chunks, with_w=False, eng=None):
                # image lands dense (full DMA bandwidth: 12.5KB contiguous
                # per partition) in a staging tile; img0 rides the ACT HWDGE
                # queue, img1 the SP queue, so neither waits on the other
                eng = eng or nc.scalar
                xs = stage_pool.tile([CIN, H, W], mm_dt, tag="stage")
                for ci, (r0, r1) in enumerate(chunks):
                    eng.dma_start(xs[:, r0:r1, :],
                                  mm_view(x_dram[img, :, r0:r1, :]))
                    if with_w and ci == 0:
                        for t in range(CO_TILES):
                            nc.scalar.dma_start(w_sb[:, :, t, :],
                                                w_dram_v[:, :, t, :])
                return xs

            def repack(xs, slot, chunks):
                # GPSIMD repacks rows into the padded layout (1-input ops
                # run at line rate there; DVE is kept free for bias-adds,
                # ACT's table-load cost makes scalar-engine copies slow)
                for r0, r1 in chunks:
                    nc.gpsimd.tensor_copy(
                        x_pads[slot][:, 1 + r0:1 + r1, 1:WP - 1],
                        xs[:, r0:r1, :])

            fixed_out = []
            if "dmaonly" in ablate:
                for s in range(2):
                    fo = out_pool.tile([128, ROWS_PER_BLOCK, W], f32,
                                       tag=f"fixo{s}")
                    nc.gpsimd.memset(fo[:].bitcast(u32), 0)
                    fixed_out.append(fo)

            def compute(img, slot):
                if "dmaonly" in ablate:
                    for rb in range(N_BLOCKS):
                        for t in range(CO_TILES):
                            h0 = rb * ROWS_PER_BLOCK
                            nc.sync.dma_start(
                                out_dram[img, t * 128:(t + 1) * 128,
                                         h0:h0 + ROWS_PER_BLOCK, :],
                                fixed_out[(rb + t) % 2][:])
                    return
                if "nomm" in ablate:
                    return
                x_pad = x_pads[slot]
                for rb in range(N_BLOCKS):
                    for t in range(CO_TILES):
                        h0 = rb * ROWS_PER_BLOCK
                        psum = psum_pool.tile([128, ROWS_PER_BLOCK, W], f32)
                        for k in range(KH * KW):
                            kh, kw = divmod(k, KW)
                            nc.tensor.matmul(
                                psum[:],
                                w_sb[:, k, t, :],
                                x_pad[:, h0 + kh:h0 + kh + ROWS_PER_BLOCK,
                                      kw:kw + W],
                                start=(k == 0),
                                stop=(k == KH * KW - 1),
                            )
                        if "nodve" in ablate:
                            continue
                        out_sb = out_pool.tile([128, ROWS_PER_BLOCK, W], f32)
                        nc.vector.tensor_scalar_add(out_sb[:], psum[:],
                                                    b_sb[:, t:t + 1])
                        if "noout" in ablate:
                            continue
                        nc.sync.dma_start(
                            out_dram[img, t * 128:(t + 1) * 128,
                                     h0:h0 + ROWS_PER_BLOCK, :],
                            out_sb[:])

            def body_chain1(do_repack):
                xs0 = trigger_load(0, WHOLE)
                if do_repack:
                    repack(xs0, 0, WHOLE)

            def body(first):
                if "chain1" in ablate:
                    return body_chain1(True)
                if "dma1" in ablate:
                    return body_chain1(False)
                # both DMA triggers go first so the ACT engine's in-order
                # wait for image 0 data cannot delay image 1's transfer
                xs0 = trigger_load(0, FIRST_CHUNKS if first else WHOLE,
                                   with_w=first)
                xs1 = trigger_load(1, WHOLE, eng=nc.sync)
                if first:
                    nc.scalar.dma_start(b_sb[:], b_dram[:])
                repack(xs0, 0, FIRST_CHUNKS if first else WHOLE)
                repack(xs1, 1, WHOLE)
                compute(0, 0)
                compute(1, 1)

            if loop_iters:
                # timing-only variant: steady-state body in a hardware loop
                for t in range(CO_TILES):
                    nc.scalar.dma_start(w_sb[:, :, t, :],
                                        w_dram_v[:, :, t, :])
                nc.scalar.dma_start(b_sb[:], b_dram[:])
                with tc.For_i(0, loop_iters, 1):
                    body(first=False)
            else:
                for r in range(repeats):
                    body(first=(r == 0))
    nc.compile()
    return nc


def _get_nc(mm_dtype_name: str, repeats: int = 1, loop_iters: int = 0,
            ablate: frozenset = frozenset()):
    key = (mm_dtype_name, repeats, loop_iters, ablate)
    if key not in _COMPILED:
        _COMPILED[key] = _build(mm_dtype_name, repeats,
                                loop_iters=loop_iters, ablate=ablate)
    return _COMPILED[key]


def prep_inputs(x, weight, bias, mm_dtype_name="float32r"):
    """Shard/transform full inputs into per-core in_maps."""
    if mm_dtype_name == "bfloat16":
        import ml_dtypes
        in_np = ml_dtypes.bfloat16
    else:
        in_np = np.float32
    x = np.ascontiguousarray(np.asarray(x, dtype=np.float32).astype(in_np))
    # [co, ci, kh, kw] -> [ci, kh, kw, t, co'] flattened to [ci, 9*256]
    w_prep = np.ascontiguousarray(
        np.asarray(weight, dtype=np.float32)
        .reshape(CO_TILES, 128, CIN, KH, KW)
        .transpose(2, 3, 4, 0, 1)
        .reshape(CIN, KH * KW * COUT).astype(in_np))
    b_prep = np.ascontiguousarray(
        bias.reshape(CO_TILES, 128).transpose(1, 0), dtype=np.float32)
    return [
        {"x": x[c * IMGS_PER_CORE:(c + 1) * IMGS_PER_CORE],
         "w": w_prep, "b": b_prep}
        for c in range(N_CORES)
    ]


def run(x, weight, bias, mm_dtype_name="float32r", trace=False):
    from concourse.bass_utils import run_bass_kernel_spmd
    nc = _get_nc(mm_dtype_name)
    in_maps = prep_inputs(x, weight, bias, mm_dtype_name)
    res = run_bass_kernel_spmd(nc, in_maps, list(range(N_CORES)), trace=trace)
    out = np.concatenate([res.results[c]["out"] for c in range(N_CORES)],
                         axis=0)
    return out, res


def kernel(x, weight, bias):
    out, _ = run(np.asarray(x), np.asarray(weight), np.asarray(bias))
    return out

